# revision 21
# baseline (speedup 1.0000x reference)
"""Trainium2 Bass kernel for nn_BOW (bag-of-words MLP).

emb = relu(relu(relu(bow(idx) @ W1.T + b1) @ W2.T + b2) @ W3.T + b3)

Design (8 NeuronCores):

fc1 is sharded over the vocab axis: core c owns rows [c*6400, (c+1)*6400) of
W1T (50000 padded to 51200).  W1 is stored as an fp8e4m3 two-term residual
split A + B (A = e4m3(S*W1), B = e4m3(S*W1 - A), shared scale S = 2^10),
which matches bf16 end-to-end accuracy while letting fc1 run in DoubleRow
fp8 perf mode: each matmul contracts TWO 128-lane vocab buckets per pass
(adjacent bucket pairs of the same table), so the dense fc1 runs at 2x bf16
MAC throughput while streaming the same 12.8 MB/core of weights.

Histogram without scatter: tokens are host-routed to (core, vocab-bucket of
128, batch-row-half) slots.  For each 128-token tile, one TensorE matmul
R^T @ M accumulates exact counts into PSUM, where R (token -> vocab lane)
and M (token -> row) are one-hot matrices built on DVE by iota-compare from
tiny per-slot id vectors (rv, rw) -- no one-hot streaming from HBM.  The
PSUM counts are copied to fp8 bowT on the Scalar engine (counts <= 15 are
exact in e4m3), feeding stage-2 DoubleRow matmuls as lhsT bucket pairs.

Partial h1 [256, 1024] (descaled by 1/S during the PSUM->bf16 copy) is
exchanged with a single AllToAll and summed on TensorE with a
stacked-identity matmul whose output is ALREADY transposed (features on
partitions), so the bf16 fc2/fc3 tail needs no PE transposes; biases fold in
via per-partition activation bias or a ones-vector matmul.  A tiny
indirect-DMA gather path (128 slots, S-scaled bf16 rows) handles
bucket-capacity overflow exactly.  Host-side prep is index routing, dtype
casts and relayout only; all model arithmetic runs on device.
"""
import os, sys

os.environ.setdefault("JAX_PLATFORMS", "cpu,axon")
try:
    import concourse.bass  # noqa: F401
except ImportError:
    sys.path.insert(0, "/opt/trn_rl_repo")

import numpy as np
import concourse.bass as bass
import concourse.tile as tile
import concourse.mybir as mybir
from concourse import bacc
from concourse.bass_utils import run_bass_kernel_spmd

N_CORES = 8
B, S = 256, 512
V = 50000
M1, M2, EMB = 1024, 512, 256
RPC = B // N_CORES           # rows per core after the exchange = 32
NB = 50                      # vocab buckets per core (50*128 = 6400)
NP = NB // 2                 # DoubleRow bucket pairs = 25
VSH = NB * 128               # vocab shard size = 6400
P_B = 256                    # token slots per (bucket, row-half) cell
TPB = 4                      # tiles per bucket = 2 halves x 2 tiles
NT = NB * TPB                # token tiles per core = 200
SPILL = 128                  # overflow token slots per core
WSCALE = 1024.0              # fp8 weight scale S (descaled at h1p copy)

_CACHE = {}


def _build(reps=1, sim=False):
    nc = bacc.Bacc("TRN2", target_bir_lowering=False, debug=False,
                   num_devices=1 if sim else N_CORES)
    f32 = mybir.dt.float32
    bf16 = mybir.dt.bfloat16
    f8 = mybir.dt.float8e4

    w1a = nc.dram_tensor("w1a", [128, NB, M1], f8, kind="ExternalInput")
    w1b = nc.dram_tensor("w1b", [128, NB, M1], f8, kind="ExternalInput")
    w1s = nc.dram_tensor("w1s", [VSH, M1], bf16, kind="ExternalInput")
    # packed f32 consts: rv | rw  -> [128, 2*NT]
    NCF = NT + NT
    cfp = nc.dram_tensor("cfp", [128, NCF], f32, kind="ExternalInput")
    msph = nc.dram_tensor("msph", [128, 256], f8, kind="ExternalInput")
    w2t = nc.dram_tensor("w2t", [128, M1 // 128, M2], bf16, kind="ExternalInput")
    w3t = nc.dram_tensor("w3t", [128, M2 // 128, EMB], bf16, kind="ExternalInput")
    # packed bf16 row consts: b1 | b2 | b3  -> [1, M1+M2+EMB]
    brow = nc.dram_tensor("brow", [1, M1 + M2 + EMB], bf16, kind="ExternalInput")
    s4id = nc.dram_tensor("s4id", [128, RPC], bf16, kind="ExternalInput")
    spidx = nc.dram_tensor("spidx", [128, 1], mybir.dt.int32, kind="ExternalInput")
    emb = nc.dram_tensor("emb", [reps * RPC, EMB], f32, kind="ExternalOutput")

    with tile.TileContext(nc) as tc:
        with (
            tc.tile_pool(name="const", bufs=1) as cpool,
            tc.tile_pool(name="etab", bufs=1) as epool,
            tc.tile_pool(name="bowsb", bufs=1) as bpool,
            tc.tile_pool(name="act", bufs=2) as apool,
            tc.tile_pool(name="oh", bufs=16) as ohpool,
            tc.tile_pool(name="pbow", bufs=2, space="PSUM") as pbow_pool,
            tc.tile_pool(name="ph1", bufs=1, space="PSUM") as ph1_pool,
            tc.tile_pool(name="ptail", bufs=1, space="PSUM") as ptail_pool,
            tc.tile_pool(name="dram", bufs=2, space="DRAM") as dpool,
        ):
            # ---- constants: rv/rw first (DVE needs them immediately);
            # everything else is emitted inside _body interleaved with the
            # weight stream so HWDGE/queue order favors the pipeline ----
            cf_t = cpool.tile([128, NCF], f32)
            nc.sync.dma_start(cf_t[:], cfp[:])
            rv_t = cf_t[:, 0:NT]
            rw_t = cf_t[:, NT:2 * NT]
            iota_i = cpool.tile([128, 128], mybir.dt.int32)
            nc.gpsimd.iota(iota_i[:], pattern=[[1, 128]], base=0, channel_multiplier=0)
            iotaR = cpool.tile([128, 128], bf16)
            nc.vector.tensor_copy(iotaR[:], iota_i[:])
            ones1 = cpool.tile([1, RPC], bf16)
            nc.gpsimd.memset(ones1[:], 1.0)
            spidx_t = cpool.tile([128, 1], mybir.dt.int32)
            nc.sync.dma_start(spidx_t[:], spidx[:])
            gsp = cpool.tile([128, M1], bf16)
            nc.gpsimd.indirect_dma_start(
                out=gsp[:], out_offset=None, in_=w1s[:],
                in_offset=bass.IndirectOffsetOnAxis(ap=spidx_t[:, 0:1], axis=0),
            )
            msp = cpool.tile([128, 256], f8)
            s4_t = cpool.tile([128, RPC], bf16)
            br_t = cpool.tile([1, M1 + M2 + EMB], bf16)
            b1_t = br_t[:, 0:M1]
            b2_t = br_t[:, M1:M1 + M2]
            b3_t = br_t[:, M1 + M2:M1 + M2 + EMB]
            w2_t = cpool.tile([128, M1 // 128, M2], bf16)
            w3_t = cpool.tile([128, M2 // 128, EMB], bf16)

            def load_consts_mid():
                nc.sync.dma_start(msp[:], msph[:])
                nc.sync.dma_start(s4_t[:], s4id[:])
                nc.sync.dma_start(br_t[:], brow[:])

            def load_consts_late():
                nc.sync.dma_start(w2_t[:], w2t[:])
                nc.sync.dma_start(w3_t[:], w3t[:])

            for _rep in range(reps):
                _body(nc, tc, epool, bpool, apool, ohpool,
                      pbow_pool, ph1_pool, ptail_pool, dpool,
                      gsp, msp, w2_t, w3_t, b1_t, b2_t, b3_t,
                      s4_t, ones1, rv_t, rw_t, iotaR,
                      emb[_rep * RPC:(_rep + 1) * RPC, :], w1a, w1b, sim,
                      load_consts_mid if _rep == 0 else None,
                      load_consts_late if _rep == 0 else None)

    nc.compile()
    return nc


def _body(nc, tc, epool, bpool, apool, ohpool,
          pbow_pool, ph1_pool, ptail_pool, dpool,
          gsp, msp, w2_t, w3_t, b1_t, b2_t, b3_t,
          s4_t, ones1, rv_t, rw_t, iotaR, emb, w1a, w1b, sim=False,
          load_consts_mid=None, load_consts_late=None):
    f32 = mybir.dt.float32
    bf16 = mybir.dt.bfloat16
    f8 = mybir.dt.float8e4
    Relu = mybir.ActivationFunctionType.Relu
    Copy = mybir.ActivationFunctionType.Copy
    DR = mybir.MatmulPerfMode.DoubleRow
    eq = mybir.AluOpType.is_equal

    # ---- stream both fp8 weight tables in big chunks (HWDGE holds each
    # DMACopy ~600ns regardless of size, so few big transfers win) ----
    etA = epool.tile([128, NB, M1], f8, tag="etA")
    etB = epool.tile([128, NB, M1], f8, tag="etB")
    CHK = 10                                   # buckets per stream DMA
    for g in range(NB // CHK):
        sl = slice(g * CHK, (g + 1) * CHK)
        nc.sync.dma_start(etA[:, sl, :], w1a[:, sl, :])
        nc.sync.dma_start(etB[:, sl, :], w1b[:, sl, :])

    # ---- stage 1 (histogram) + stage 2 (DoubleRow fc1), single pass ----
    bowT = bpool.tile([128, NB, 256], f8, tag="bowT")
    ph1 = ph1_pool.tile([128, 2, 2, 512], f32, tag="h1")

    def stage1(q):
        pb = pbow_pool.tile([128, 256], f32, tag="bow")
        for j in range(TPB):
            t = q * TPB + j
            h = j // 2
            rt = ohpool.tile([128, 128], bf16, tag="oh")
            nc.vector.tensor_scalar(rt[:], iotaR[:], rv_t[:, t:t + 1], None, op0=eq)
            rm = ohpool.tile([128, 128], bf16, tag="oh")
            nc.vector.tensor_scalar(rm[:], iotaR[:], rw_t[:, t:t + 1], None, op0=eq)
            nc.tensor.matmul(pb[:, h * 128:(h + 1) * 128], lhsT=rt[:], rhs=rm[:],
                             start=(j % 2 == 0), stop=(j % 2 == 1))
        nc.scalar.activation(bowT[:, q, :], pb[:], Copy)

    def stage2(p):
        for h in range(2):
            for m in range(2):
                nc.tensor.matmul(
                    ph1[:, h, m, :],
                    lhsT=bowT[:, 2 * p:2 * p + 2, h * 128:(h + 1) * 128],
                    rhs=etA[:, 2 * p:2 * p + 2, m * 512:(m + 1) * 512],
                    start=(p == 0), stop=False, perf_mode=DR)
                nc.tensor.matmul(
                    ph1[:, h, m, :],
                    lhsT=bowT[:, 2 * p:2 * p + 2, h * 128:(h + 1) * 128],
                    rhs=etB[:, 2 * p:2 * p + 2, m * 512:(m + 1) * 512],
                    start=False, stop=False, perf_mode=DR)

    for p in range(NP + 1):
        if p < NP:
            stage1(2 * p)
            stage1(2 * p + 1)
        if p >= 1:
            stage2(p - 1)
    # spill contribution (S-scaled bf16 rows, exact) ends the accumulation
    for h in range(2):
        for m in range(2):
            nc.tensor.matmul(ph1[:, h, m, :], lhsT=msp[:, h * 128:(h + 1) * 128],
                             rhs=gsp[:, m * 512:(m + 1) * 512],
                             start=False, stop=(h == 1 and m == 1))
    # PSUM -> bf16 cast copies (still S-scaled; 1/S folds into s4 selector).
    # Split across Scalar + DVE so the two engines cast in parallel.
    h1p = apool.tile([128, 2, 2, 512], bf16, tag="h1p")
    nc.scalar.activation(h1p[:, 0, 0, :], ph1[:, 0, 0, :], Copy)
    nc.vector.tensor_copy(h1p[:, 0, 1, :], ph1[:, 0, 1, :])
    nc.scalar.activation(h1p[:, 1, 0, :], ph1[:, 1, 0, :], Copy)
    nc.vector.tensor_copy(h1p[:, 1, 1, :], ph1[:, 1, 1, :])

    # ---- exchange partial h1: AllToAll (8 chunks of 32 rows) ----
    cc_in = dpool.tile([B, M1], bf16, tag="cc_in")
    cc_out = dpool.tile([B, M1], bf16, tag="cc_out")
    nc.sync.dma_start(
        cc_in[:].rearrange("(h p) (b m) -> p h b m", p=128, b=2), h1p[:])
    if sim:
        nc.sync.dma_start(cc_out[:], cc_in[:])
    else:
        nc.gpsimd.collective_compute(
            "AllToAll", mybir.AluOpType.bypass,
            replica_groups=[list(range(N_CORES))],
            ins=[cc_in[:]], outs=[cc_out[:]],
        )
    cc_sb = apool.tile([128, 2, M1], bf16, tag="ccsb")
    nc.sync.dma_start(
        cc_sb[:], cc_out[:].rearrange("(d q r) m -> (q r) d m", d=2, q=4))

    # ---- sum the 8 partials on TensorE, TRANSPOSED: h1T [feat128, a, rows];
    # s4 selector carries 1/S; b1 folds in via a K=1 ones-matmul so the
    # relu is ONE wide activation, not 8 per-bias ops ----
    pt1 = ptail_pool.tile([128, M1 // 128, RPC], f32, tag="tail")
    for a in range(M1 // 128):
        for d in range(2):
            nc.tensor.matmul(pt1[:, a, :], lhsT=cc_sb[:, d, a * 128:(a + 1) * 128],
                             rhs=s4_t[:], start=(d == 0), stop=False)
        nc.tensor.matmul(pt1[:, a, :], lhsT=b1_t[:, a * 128:(a + 1) * 128],
                         rhs=ones1[:], start=False, stop=True)
    h1T = apool.tile([128, M1 // 128, RPC], bf16, tag="h1T")
    nc.scalar.activation(h1T[:], pt1[:], Relu)

    # ---- fc2, output transposed: h2T [feat128, m4, rows] ----
    pt2 = ptail_pool.tile([128, M1 // 128, RPC], f32, tag="tail")
    for m4 in range(M2 // 128):
        for a in range(M1 // 128):
            nc.tensor.matmul(pt2[:, m4, :], lhsT=w2_t[:, a, m4 * 128:(m4 + 1) * 128],
                             rhs=h1T[:, a, :],
                             start=(a == 0), stop=False)
        nc.tensor.matmul(pt2[:, m4, :], lhsT=b2_t[:, m4 * 128:(m4 + 1) * 128],
                         rhs=ones1[:], start=False, stop=True)
    h2T = apool.tile([128, M2 // 128, RPC], bf16, tag="h2T")
    nc.scalar.activation(h2T[:], pt2[:, 0:M2 // 128, :], Relu)

    # ---- fc3, row-major output [32, 256] ----
    pt3f = ptail_pool.tile([128, M1 // 128, RPC], f32, tag="tail")
    pt3 = pt3f[0:RPC, 0:EMB // RPC, :]
    for m4 in range(M2 // 128):
        nc.tensor.matmul(pt3, lhsT=h2T[:, m4, :], rhs=w3_t[:, m4, :],
                         start=(m4 == 0), stop=False)
    nc.tensor.matmul(pt3, lhsT=ones1[:], rhs=b3_t[:], start=False, stop=True)
    out_t = apool.tile([RPC, EMB], f32, tag="out")
    nc.scalar.activation(out_t[:], pt3, Relu)
    nc.sync.dma_start(emb[:], out_t[:])


def _prep_inputs(idx, W1, b1, W2, b2, W3, b3):
    """Host-side sharding/layout prep (index routing + dtype/layout only)."""
    import ml_dtypes

    bf16 = ml_dtypes.bfloat16
    f8np = mybir.dt.np(mybir.dt.float8e4)
    idx = np.asarray(idx).astype(np.int64)
    VPAD = N_CORES * VSH
    w1f = np.zeros((VPAD, M1), dtype=np.float32)
    w1f[:V] = np.asarray(W1, dtype=np.float32).T
    w1f *= WSCALE
    w1A = w1f.astype(f8np)
    w1B = (w1f - w1A.astype(np.float32)).astype(f8np)
    w1sc = w1f.astype(bf16)          # S-scaled bf16 rows for the spill gather

    w2t = np.ascontiguousarray(
        np.asarray(W2, dtype=np.float32).T.reshape(M1 // 128, 128, M2)
        .transpose(1, 0, 2)).astype(bf16)
    w3t = np.ascontiguousarray(
        np.asarray(W3, dtype=np.float32).T.reshape(M2 // 128, 128, EMB)
        .transpose(1, 0, 2)).astype(bf16)
    browp = np.concatenate([
        np.asarray(b1, dtype=np.float32),
        np.asarray(b2, dtype=np.float32),
        np.asarray(b3, dtype=np.float32)]).reshape(1, -1).astype(bf16)
    s4id = ((np.arange(128)[:, None] % RPC == np.arange(RPC)[None, :])
            .astype(np.float32) / WSCALE).astype(bf16)

    rows = np.repeat(np.arange(B, dtype=np.int64), S)
    vals = idx.reshape(-1)
    core = vals // VSH
    in_maps = []
    for c in range(N_CORES):
        sel = core == c
        v = vals[sel] - c * VSH
        r = rows[sel]
        q = v // 128
        rl = v % 128
        order = np.argsort(q, kind="stable")
        q, rl, r, v = q[order], rl[order], r[order], v[order]

        rv_arr = np.full((NT * 128,), 200, dtype=np.int64)
        rw_arr = np.full((NT * 128,), 300, dtype=np.int64)
        sp_idx = np.zeros((SPILL,), dtype=np.int32)
        sp_row = np.full((SPILL,), 300, dtype=np.int64)
        n_spill = 0
        for qq in range(NB):
            for hh in range(2):
                m = (q == qq) & ((r // 128) == hh)
                nq = int(m.sum())
                take = min(nq, P_B)
                base = (qq * 4 + hh * 2) * 128
                rv_arr[base:base + take] = rl[m][:take]
                rw_arr[base:base + take] = r[m][:take] % 128
                if nq > take:
                    ov = nq - take
                    assert n_spill + ov <= SPILL, "spill capacity exceeded"
                    sp_idx[n_spill:n_spill + ov] = v[m][take:]
                    sp_row[n_spill:n_spill + ov] = r[m][take:]
                    n_spill += ov
        rv_til = rv_arr.reshape(NT, 128).T        # [128, NT]
        rw_til = rw_arr.reshape(NT, 128).T
        cfpk = np.ascontiguousarray(np.concatenate(
            [rv_til, rw_til], axis=1).astype(np.float32))
        mspa = (sp_row[:, None] == np.arange(256)[None, :]).astype(f8np)

        w1Ac = w1A[c * VSH:(c + 1) * VSH]                     # [6400, 1024]
        w1Bc = w1B[c * VSH:(c + 1) * VSH]
        w1atl = np.ascontiguousarray(
            w1Ac.reshape(NB, 128, M1).transpose(1, 0, 2))     # [128, 50, 1024]
        w1btl = np.ascontiguousarray(
            w1Bc.reshape(NB, 128, M1).transpose(1, 0, 2))

        in_maps.append({
            "w1a": w1atl,
            "w1b": w1btl,
            "w1s": np.ascontiguousarray(w1sc[c * VSH:(c + 1) * VSH]),
            "cfp": cfpk,
            "msph": mspa,
            "w2t": w2t, "w3t": w3t,
            "brow": browp, "s4id": s4id,
            "spidx": sp_idx.reshape(128, 1),
        })
    return in_maps


def kernel(idx, W1, b1, W2, b2, W3, b3):
    if "nc" not in _CACHE:
        _CACHE["nc"] = _build()
    nc = _CACHE["nc"]
    in_maps = _prep_inputs(idx, W1, b1, W2, b2, W3, b3)
    try:
        res = run_bass_kernel_spmd(nc, in_maps, list(range(N_CORES)))
    except Exception:
        res = run_bass_kernel_spmd(nc, in_maps, list(range(N_CORES)))
    return np.concatenate([res.results[c]["emb"] for c in range(N_CORES)], axis=0)


# revision 36
# speedup vs baseline: 1.0308x; 1.0308x over previous
"""Trainium2 Bass kernel for nn_BOW (bag-of-words MLP).

emb = relu(relu(relu(bow(idx) @ W1.T + b1) @ W2.T + b2) @ W3.T + b3)

Design (8 NeuronCores):

fc1 is sharded over the vocab axis: core c owns rows [c*6400, (c+1)*6400) of
W1T (50000 padded to 51200).  W1 is stored as an fp8e4m3 two-term residual
split A + B (A = e4m3(S*W1), B = e4m3(S*W1 - A), shared scale S = 2^10),
which matches bf16 end-to-end accuracy while letting fc1 run in DoubleRow
fp8 perf mode: each matmul contracts TWO 128-lane vocab buckets per pass
(adjacent bucket pairs of the same table), so the dense fc1 runs at 2x bf16
MAC throughput while streaming the same 12.8 MB/core of weights.

Histogram without scatter: tokens are host-routed to (core, vocab-bucket of
128, batch-row-half) slots.  For each 128-token tile, one TensorE matmul
R^T @ M accumulates exact counts into PSUM, where R (token -> vocab lane)
and M (token -> row) are one-hot matrices built on DVE by iota-compare from
tiny per-slot id vectors (rv, rw) -- no one-hot streaming from HBM.  The
PSUM counts are copied to fp8 bowT on the Scalar engine (counts <= 15 are
exact in e4m3), feeding stage-2 DoubleRow matmuls as lhsT bucket pairs.

Partial h1 [256, 1024] (S-scaled bf16; 1/S folds into the stacked-identity
selector) is exchanged with a single AllToAll and summed on TensorE with a
matmul whose output is ALREADY transposed (features on partitions), so the
bf16 fc2/fc3 tail needs no PE transposes; b1/b2/b3 fold in via K=1
ones-vector matmuls so each relu is one wide activation.  A tiny
indirect-DMA gather path (128 slots, S-scaled bf16 rows) handles
bucket-capacity overflow exactly.  Host-side prep is index routing, dtype
casts and relayout only; all model arithmetic runs on device.

Scheduling: engines execute their queues IN ORDER, so rep N's tail
(exchange + fc2/fc3) is emitted one rep late and its DMA legs ride the
Pool/SWDGE queue -- no tail instruction ever sits between two reps' main
work on the SP/DVE/Act/PE queues (head-of-line blocking there serialized
whole reps: measured +65 us/rep).  Stream DMAs are issued in 640 KB chunks
because the shared HWDGE descriptor unit costs ~625 ns per DMACopy
regardless of size.
"""
import os, sys

os.environ.setdefault("JAX_PLATFORMS", "cpu,axon")
try:
    import concourse.bass  # noqa: F401
except ImportError:
    sys.path.insert(0, "/opt/trn_rl_repo")

import numpy as np
import concourse.bass as bass
import concourse.tile as tile
import concourse.mybir as mybir
from concourse import bacc
from concourse.bass_utils import run_bass_kernel_spmd

N_CORES = 8
B, S = 256, 512
V = 50000
M1, M2, EMB = 1024, 512, 256
RPC = B // N_CORES           # rows per core after the exchange = 32
NB = 50                      # vocab buckets per core (50*128 = 6400)
NP = NB // 2                 # DoubleRow bucket pairs = 25
VSH = NB * 128               # vocab shard size = 6400
P_B = 256                    # token slots per (bucket, row-half) cell
TPB = 4                      # tiles per bucket = 2 halves x 2 tiles
NT = NB * TPB                # token tiles per core = 200
SPILL = 128                  # overflow token slots per core
WSCALE = 1024.0              # fp8 weight scale S (descaled at h1p copy)

_CACHE = {}


def _build(reps=1, sim=False, nocc=False, ablate=None):
    # sim: single-core build with the collective stubbed (for TimelineSim).
    # nocc: 8-core build with the collective stubbed (HW contention probe).
    # ablate: None | 'stream' | 'stage1' | 'nos2tail' (HW bottleneck probes).
    nc = bacc.Bacc("TRN2", target_bir_lowering=False, debug=False,
                   num_devices=1 if sim else N_CORES)
    sim = sim or nocc
    f32 = mybir.dt.float32
    bf16 = mybir.dt.bfloat16
    f8 = mybir.dt.float8e4

    w1a = nc.dram_tensor("w1a", [128, NB, M1], f8, kind="ExternalInput")
    w1b = nc.dram_tensor("w1b", [128, NB, M1], f8, kind="ExternalInput")
    w1s = nc.dram_tensor("w1s", [VSH, M1], bf16, kind="ExternalInput")
    # packed f32 consts: rv | rw  -> [128, 2*NT]
    NCF = NT + NT
    cfp = nc.dram_tensor("cfp", [128, NCF], f32, kind="ExternalInput")
    msph = nc.dram_tensor("msph", [128, 256], f8, kind="ExternalInput")
    w2t = nc.dram_tensor("w2t", [128, M1 // 128, M2], bf16, kind="ExternalInput")
    w3t = nc.dram_tensor("w3t", [128, M2 // 128, EMB], bf16, kind="ExternalInput")
    # packed bf16 row consts: b1 | b2 | b3  -> [1, M1+M2+EMB]
    brow = nc.dram_tensor("brow", [1, M1 + M2 + EMB], bf16, kind="ExternalInput")
    s4id = nc.dram_tensor("s4id", [128, RPC], bf16, kind="ExternalInput")
    spidx = nc.dram_tensor("spidx", [128, 1], mybir.dt.int32, kind="ExternalInput")
    emb = nc.dram_tensor("emb", [reps * RPC, EMB], f32, kind="ExternalOutput")

    with tile.TileContext(nc) as tc:
        with (
            tc.tile_pool(name="const", bufs=1) as cpool,
            tc.tile_pool(name="etab", bufs=1) as epool,
            tc.tile_pool(name="bowsb", bufs=2) as bpool,
            tc.tile_pool(name="act", bufs=2) as apool,
            tc.tile_pool(name="oh", bufs=16) as ohpool,
            tc.tile_pool(name="pbow", bufs=2, space="PSUM") as pbow_pool,
            tc.tile_pool(name="ph1", bufs=1, space="PSUM") as ph1_pool,
            tc.tile_pool(name="ptail", bufs=1, space="PSUM") as ptail_pool,
            tc.tile_pool(name="dram", bufs=2, space="DRAM") as dpool,
        ):
            # ---- constants: rv/rw first (DVE needs them immediately);
            # everything else is emitted inside _body interleaved with the
            # weight stream so HWDGE/queue order favors the pipeline ----
            cf_t = cpool.tile([128, NCF], f32)
            nc.sync.dma_start(cf_t[:], cfp[:])
            rv_t = cf_t[:, 0:NT]
            rw_t = cf_t[:, NT:2 * NT]
            iota_i = cpool.tile([128, 128], mybir.dt.int32)
            nc.gpsimd.iota(iota_i[:], pattern=[[1, 128]], base=0, channel_multiplier=0)
            iotaR = cpool.tile([128, 128], bf16)
            nc.vector.tensor_copy(iotaR[:], iota_i[:])
            ones1 = cpool.tile([1, RPC], bf16)
            nc.gpsimd.memset(ones1[:], 1.0)
            spidx_t = cpool.tile([128, 1], mybir.dt.int32)
            nc.sync.dma_start(spidx_t[:], spidx[:])
            gsp = cpool.tile([128, M1], bf16)
            nc.gpsimd.indirect_dma_start(
                out=gsp[:], out_offset=None, in_=w1s[:],
                in_offset=bass.IndirectOffsetOnAxis(ap=spidx_t[:, 0:1], axis=0),
            )
            msp = cpool.tile([128, 256], f8)
            s4_t = cpool.tile([128, RPC], bf16)
            br_t = cpool.tile([1, M1 + M2 + EMB], bf16)
            b1_t = br_t[:, 0:M1]
            b2_t = br_t[:, M1:M1 + M2]
            b3_t = br_t[:, M1 + M2:M1 + M2 + EMB]
            w2_t = cpool.tile([128, M1 // 128, M2], bf16)
            w3_t = cpool.tile([128, M2 // 128, EMB], bf16)

            def load_consts_mid():
                nc.sync.dma_start(msp[:], msph[:])
                nc.sync.dma_start(s4_t[:], s4id[:])
                nc.sync.dma_start(br_t[:], brow[:])

            def load_consts_late():
                nc.sync.dma_start(w2_t[:], w2t[:])
                nc.sync.dma_start(w3_t[:], w3t[:])

            # 1-rep software pipeline: rep N's tail (collective + fc2/fc3)
            # is EMITTED after rep N+1's main, so no engine queue has a
            # tail instruction blocking the next rep's main work
            # (head-of-line ordering is what serialized reps on HW).
            pend = None
            for _rep in range(reps):
                h1p = _main(nc, tc, epool, bpool, apool, ohpool,
                            pbow_pool, ph1_pool,
                            gsp, msp, rv_t, rw_t, iotaR,
                            w1a, w1b,
                            load_consts_mid if _rep == 0 else None,
                            load_consts_late if _rep == 0 else None,
                            ablate)
                if pend is not None:
                    _tail(nc, apool, ptail_pool, dpool, pend,
                          w2_t, w3_t, b1_t, b2_t, b3_t, s4_t, ones1,
                          emb[(_rep - 1) * RPC:_rep * RPC, :], sim)
                if h1p is None and ablate is not None:
                    # ablation mode: dummy output, no tail
                    dummy = apool.tile([RPC, EMB], f32, tag="out")
                    nc.gpsimd.memset(dummy[:], 0.0)
                    nc.sync.dma_start(emb[_rep * RPC:(_rep + 1) * RPC, :],
                                      dummy[:])
                pend = h1p
            if pend is not None:
                _tail(nc, apool, ptail_pool, dpool, pend,
                      w2_t, w3_t, b1_t, b2_t, b3_t, s4_t, ones1,
                      emb[(reps - 1) * RPC:reps * RPC, :], sim)

    nc.compile()
    return nc


def _main(nc, tc, epool, bpool, apool, ohpool, pbow_pool, ph1_pool,
          gsp, msp, rv_t, rw_t, iotaR, w1a, w1b,
          load_consts_mid=None, load_consts_late=None, ablate=None):
    """Stream + histogram + DoubleRow fc1 + PSUM->bf16 casts.

    Returns the h1p tile (S-scaled bf16 partial h1), or None in ablation
    modes that stop early.
    """
    f32 = mybir.dt.float32
    bf16 = mybir.dt.bfloat16
    f8 = mybir.dt.float8e4
    Copy = mybir.ActivationFunctionType.Copy
    DR = mybir.MatmulPerfMode.DoubleRow
    eq = mybir.AluOpType.is_equal

    # ---- stream both fp8 weight tables in big chunks (HWDGE holds each
    # DMACopy ~600ns regardless of size, so few big transfers win);
    # small consts slot in after the first chunk, w2/w3 after the stream ----
    etA = epool.tile([128, NB, M1], f8, tag="etA")
    etB = epool.tile([128, NB, M1], f8, tag="etB")
    CHK = 5                                    # buckets per stream DMA
    for g in range(NB // CHK):
        sl = slice(g * CHK, (g + 1) * CHK)
        nc.sync.dma_start(etA[:, sl, :], w1a[:, sl, :])
        nc.sync.dma_start(etB[:, sl, :], w1b[:, sl, :])
        if g == 0 and load_consts_mid is not None:
            load_consts_mid()
    if load_consts_late is not None:
        load_consts_late()

    # ---- stage 1 (histogram) + stage 2 (DoubleRow fc1), single pass ----
    bowT = bpool.tile([128, NB, 256], f8, tag="bowT")
    ph1 = ph1_pool.tile([128, 2, 2, 512], f32, tag="h1")

    def stage1(q):
        pb = pbow_pool.tile([128, 256], f32, tag="bow")
        for j in range(TPB):
            t = q * TPB + j
            h = j // 2
            rt = ohpool.tile([128, 128], bf16, tag="oh")
            nc.vector.tensor_scalar(rt[:], iotaR[:], rv_t[:, t:t + 1], None, op0=eq)
            rm = ohpool.tile([128, 128], bf16, tag="oh")
            nc.vector.tensor_scalar(rm[:], iotaR[:], rw_t[:, t:t + 1], None, op0=eq)
            nc.tensor.matmul(pb[:, h * 128:(h + 1) * 128], lhsT=rt[:], rhs=rm[:],
                             start=(j % 2 == 0), stop=(j % 2 == 1))
        nc.scalar.activation(bowT[:, q, :], pb[:], Copy)

    def stage2(p):
        for h in range(2):
            for m in range(2):
                nc.tensor.matmul(
                    ph1[:, h, m, :],
                    lhsT=bowT[:, 2 * p:2 * p + 2, h * 128:(h + 1) * 128],
                    rhs=etA[:, 2 * p:2 * p + 2, m * 512:(m + 1) * 512],
                    start=(p == 0), stop=False, perf_mode=DR)
                nc.tensor.matmul(
                    ph1[:, h, m, :],
                    lhsT=bowT[:, 2 * p:2 * p + 2, h * 128:(h + 1) * 128],
                    rhs=etB[:, 2 * p:2 * p + 2, m * 512:(m + 1) * 512],
                    start=False, stop=False, perf_mode=DR)

    if ablate == "stream":
        return None
    for p in range(NP):
        stage1(2 * p)
        stage1(2 * p + 1)
        if p >= 1 and ablate != "stage1":
            stage2(p - 1)
    if ablate == "stage1":
        return None
    # final pair + spill, finished REGION BY REGION so the PSUM -> bf16 cast
    # copies (split across Scalar + DVE; 1/S folds into the s4 selector)
    # overlap the remaining matmuls
    h1p = apool.tile([128, 2, 2, 512], bf16, tag="h1p")
    pl = NP - 1
    for i, (h, m) in enumerate(((0, 0), (0, 1), (1, 0), (1, 1))):
        nc.tensor.matmul(
            ph1[:, h, m, :],
            lhsT=bowT[:, 2 * pl:2 * pl + 2, h * 128:(h + 1) * 128],
            rhs=etA[:, 2 * pl:2 * pl + 2, m * 512:(m + 1) * 512],
            start=False, stop=False, perf_mode=DR)
        nc.tensor.matmul(
            ph1[:, h, m, :],
            lhsT=bowT[:, 2 * pl:2 * pl + 2, h * 128:(h + 1) * 128],
            rhs=etB[:, 2 * pl:2 * pl + 2, m * 512:(m + 1) * 512],
            start=False, stop=False, perf_mode=DR)
        # spill contribution (S-scaled bf16 rows, exact) ends this region
        nc.tensor.matmul(ph1[:, h, m, :], lhsT=msp[:, h * 128:(h + 1) * 128],
                         rhs=gsp[:, m * 512:(m + 1) * 512],
                         start=False, stop=True)
        if i % 2 == 0:
            nc.scalar.activation(h1p[:, h, m, :], ph1[:, h, m, :], Copy)
        else:
            nc.vector.tensor_copy(h1p[:, h, m, :], ph1[:, h, m, :])
    if ablate == "nos2tail":
        return None
    return h1p


def _tail(nc, apool, ptail_pool, dpool, h1p,
          w2_t, w3_t, b1_t, b2_t, b3_t, s4_t, ones1, emb, sim=False):
    """Exchange + partial-sum + fc2/fc3.  All DMA legs ride the Pool queue
    (alongside the collective) so the SP stream queue never blocks on a
    collective, and tail matmuls are emitted one rep late (see _build)."""
    f32 = mybir.dt.float32
    bf16 = mybir.dt.bfloat16
    Relu = mybir.ActivationFunctionType.Relu

    # ---- exchange partial h1: AllToAll (8 chunks of 32 rows) ----
    cc_in = dpool.tile([B, M1], bf16, tag="cc_in")
    cc_out = dpool.tile([B, M1], bf16, tag="cc_out")
    ccv = cc_in[:].rearrange("(h p) (b m) -> h p b m", p=128, b=2)
    nc.gpsimd.dma_start(ccv[0], h1p[:, 0])
    nc.gpsimd.dma_start(ccv[1], h1p[:, 1])
    if sim:
        nc.gpsimd.dma_start(cc_out[:], cc_in[:])
    else:
        nc.gpsimd.collective_compute(
            "AllToAll", mybir.AluOpType.bypass,
            replica_groups=[list(range(N_CORES))],
            ins=[cc_in[:]], outs=[cc_out[:]],
        )
    cc_sb = apool.tile([128, 2, M1], bf16, tag="ccsb")
    nc.gpsimd.dma_start(
        cc_sb[:], cc_out[:].rearrange("(d q r) m -> (q r) d m", d=2, q=4))

    # ---- sum the 8 partials on TensorE, TRANSPOSED: h1T [feat128, a, rows];
    # s4 selector carries 1/S; b1 folds in via a K=1 ones-matmul so the
    # relu is ONE wide activation, not 8 per-bias ops ----
    pt1 = ptail_pool.tile([128, M1 // 128, RPC], f32, tag="tail")
    for a in range(M1 // 128):
        for d in range(2):
            nc.tensor.matmul(pt1[:, a, :], lhsT=cc_sb[:, d, a * 128:(a + 1) * 128],
                             rhs=s4_t[:], start=(d == 0), stop=False)
        nc.tensor.matmul(pt1[:, a, :], lhsT=b1_t[:, a * 128:(a + 1) * 128],
                         rhs=ones1[:], start=False, stop=True)
    h1T = apool.tile([128, M1 // 128, RPC], bf16, tag="h1T")
    nc.scalar.activation(h1T[:], pt1[:], Relu)

    # ---- fc2, output transposed: h2T [feat128, m4, rows] ----
    pt2 = ptail_pool.tile([128, M1 // 128, RPC], f32, tag="tail")
    for m4 in range(M2 // 128):
        for a in range(M1 // 128):
            nc.tensor.matmul(pt2[:, m4, :], lhsT=w2_t[:, a, m4 * 128:(m4 + 1) * 128],
                             rhs=h1T[:, a, :],
                             start=(a == 0), stop=False)
        nc.tensor.matmul(pt2[:, m4, :], lhsT=b2_t[:, m4 * 128:(m4 + 1) * 128],
                         rhs=ones1[:], start=False, stop=True)
    h2T = apool.tile([128, M2 // 128, RPC], bf16, tag="h2T")
    nc.scalar.activation(h2T[:], pt2[:, 0:M2 // 128, :], Relu)

    # ---- fc3, row-major output [32, 256] ----
    pt3f = ptail_pool.tile([128, M1 // 128, RPC], f32, tag="tail")
    pt3 = pt3f[0:RPC, 0:EMB // RPC, :]
    for m4 in range(M2 // 128):
        nc.tensor.matmul(pt3, lhsT=h2T[:, m4, :], rhs=w3_t[:, m4, :],
                         start=(m4 == 0), stop=False)
    nc.tensor.matmul(pt3, lhsT=ones1[:], rhs=b3_t[:], start=False, stop=True)
    out_t = apool.tile([RPC, EMB], f32, tag="out")
    nc.scalar.activation(out_t[:], pt3, Relu)
    nc.gpsimd.dma_start(emb[:], out_t[:])


def _prep_inputs(idx, W1, b1, W2, b2, W3, b3):
    """Host-side sharding/layout prep (index routing + dtype/layout only)."""
    import ml_dtypes

    bf16 = ml_dtypes.bfloat16
    f8np = mybir.dt.np(mybir.dt.float8e4)
    idx = np.asarray(idx).astype(np.int64)
    VPAD = N_CORES * VSH
    w1f = np.zeros((VPAD, M1), dtype=np.float32)
    w1f[:V] = np.asarray(W1, dtype=np.float32).T
    w1f *= WSCALE
    w1A = w1f.astype(f8np)
    w1B = (w1f - w1A.astype(np.float32)).astype(f8np)
    w1sc = w1f.astype(bf16)          # S-scaled bf16 rows for the spill gather

    w2t = np.ascontiguousarray(
        np.asarray(W2, dtype=np.float32).T.reshape(M1 // 128, 128, M2)
        .transpose(1, 0, 2)).astype(bf16)
    w3t = np.ascontiguousarray(
        np.asarray(W3, dtype=np.float32).T.reshape(M2 // 128, 128, EMB)
        .transpose(1, 0, 2)).astype(bf16)
    browp = np.concatenate([
        np.asarray(b1, dtype=np.float32),
        np.asarray(b2, dtype=np.float32),
        np.asarray(b3, dtype=np.float32)]).reshape(1, -1).astype(bf16)
    s4id = ((np.arange(128)[:, None] % RPC == np.arange(RPC)[None, :])
            .astype(np.float32) / WSCALE).astype(bf16)

    rows = np.repeat(np.arange(B, dtype=np.int64), S)
    vals = idx.reshape(-1)
    core = vals // VSH
    in_maps = []
    for c in range(N_CORES):
        sel = core == c
        v = vals[sel] - c * VSH
        r = rows[sel]
        q = v // 128
        rl = v % 128
        order = np.argsort(q, kind="stable")
        q, rl, r, v = q[order], rl[order], r[order], v[order]

        rv_arr = np.full((NT * 128,), 200, dtype=np.int64)
        rw_arr = np.full((NT * 128,), 300, dtype=np.int64)
        sp_idx = np.zeros((SPILL,), dtype=np.int32)
        sp_row = np.full((SPILL,), 300, dtype=np.int64)
        n_spill = 0
        for qq in range(NB):
            for hh in range(2):
                m = (q == qq) & ((r // 128) == hh)
                nq = int(m.sum())
                take = min(nq, P_B)
                base = (qq * 4 + hh * 2) * 128
                rv_arr[base:base + take] = rl[m][:take]
                rw_arr[base:base + take] = r[m][:take] % 128
                if nq > take:
                    ov = nq - take
                    assert n_spill + ov <= SPILL, "spill capacity exceeded"
                    sp_idx[n_spill:n_spill + ov] = v[m][take:]
                    sp_row[n_spill:n_spill + ov] = r[m][take:]
                    n_spill += ov
        rv_til = rv_arr.reshape(NT, 128).T        # [128, NT]
        rw_til = rw_arr.reshape(NT, 128).T
        cfpk = np.ascontiguousarray(np.concatenate(
            [rv_til, rw_til], axis=1).astype(np.float32))
        mspa = (sp_row[:, None] == np.arange(256)[None, :]).astype(f8np)

        w1Ac = w1A[c * VSH:(c + 1) * VSH]                     # [6400, 1024]
        w1Bc = w1B[c * VSH:(c + 1) * VSH]
        w1atl = np.ascontiguousarray(
            w1Ac.reshape(NB, 128, M1).transpose(1, 0, 2))     # [128, 50, 1024]
        w1btl = np.ascontiguousarray(
            w1Bc.reshape(NB, 128, M1).transpose(1, 0, 2))

        in_maps.append({
            "w1a": w1atl,
            "w1b": w1btl,
            "w1s": np.ascontiguousarray(w1sc[c * VSH:(c + 1) * VSH]),
            "cfp": cfpk,
            "msph": mspa,
            "w2t": w2t, "w3t": w3t,
            "brow": browp, "s4id": s4id,
            "spidx": sp_idx.reshape(128, 1),
        })
    return in_maps


def kernel(idx, W1, b1, W2, b2, W3, b3):
    if "nc" not in _CACHE:
        _CACHE["nc"] = _build()
    nc = _CACHE["nc"]
    in_maps = _prep_inputs(idx, W1, b1, W2, b2, W3, b3)
    try:
        res = run_bass_kernel_spmd(nc, in_maps, list(range(N_CORES)))
    except Exception:
        res = run_bass_kernel_spmd(nc, in_maps, list(range(N_CORES)))
    return np.concatenate([res.results[c]["emb"] for c in range(N_CORES)], axis=0)


# revision 37
# speedup vs baseline: 1.0508x; 1.0194x over previous
"""Trainium2 Bass kernel for nn_BOW (bag-of-words MLP).

emb = relu(relu(relu(bow(idx) @ W1.T + b1) @ W2.T + b2) @ W3.T + b3)

Design (8 NeuronCores):

fc1 is sharded over the vocab axis: core c owns rows [c*6400, (c+1)*6400) of
W1T (50000 padded to 51200).  W1 is stored as an fp8e4m3 two-term residual
split A + B (A = e4m3(S*W1), B = e4m3(S*W1 - A), shared scale S = 2^10),
which matches bf16 end-to-end accuracy while letting fc1 run in DoubleRow
fp8 perf mode: each matmul contracts TWO 128-lane vocab buckets per pass
(adjacent bucket pairs of the same table), so the dense fc1 runs at 2x bf16
MAC throughput while streaming the same 12.8 MB/core of weights.

Histogram without scatter: tokens are host-routed to (core, vocab-bucket of
128, batch-row-half) slots.  For each 128-token tile, one TensorE matmul
R^T @ M accumulates exact counts into PSUM, where R (token -> vocab lane)
and M (token -> row) are one-hot matrices built on DVE by iota-compare from
tiny per-slot id vectors (rv, rw) -- no one-hot streaming from HBM.  The
PSUM counts are copied to fp8 bowT on the Scalar engine (counts <= 15 are
exact in e4m3), feeding stage-2 DoubleRow matmuls as lhsT bucket pairs.

Partial h1 [256, 1024] (S-scaled bf16; 1/S folds into the stacked-identity
selector) is exchanged with a single AllToAll and summed on TensorE with a
matmul whose output is ALREADY transposed (features on partitions), so the
bf16 fc2/fc3 tail needs no PE transposes; b1/b2/b3 fold in via K=1
ones-vector matmuls so each relu is one wide activation.  A tiny
indirect-DMA gather path (128 slots, S-scaled bf16 rows) handles
bucket-capacity overflow exactly.  Host-side prep is index routing, dtype
casts and relayout only; all model arithmetic runs on device.

Scheduling: engines execute their queues IN ORDER, so rep N's tail
(exchange + fc2/fc3) is emitted one rep late and its DMA legs ride the
Pool/SWDGE queue -- no tail instruction ever sits between two reps' main
work on the SP/DVE/Act/PE queues (head-of-line blocking there serialized
whole reps: measured +65 us/rep).  Stream DMAs are issued in 640 KB chunks
because the shared HWDGE descriptor unit costs ~625 ns per DMACopy
regardless of size.
"""
import os, sys

os.environ.setdefault("JAX_PLATFORMS", "cpu,axon")
try:
    import concourse.bass  # noqa: F401
except ImportError:
    sys.path.insert(0, "/opt/trn_rl_repo")

import numpy as np
import concourse.bass as bass
import concourse.tile as tile
import concourse.mybir as mybir
from concourse import bacc
from concourse.bass_utils import run_bass_kernel_spmd

N_CORES = 8
B, S = 256, 512
V = 50000
M1, M2, EMB = 1024, 512, 256
RPC = B // N_CORES           # rows per core after the exchange = 32
NB = 50                      # vocab buckets per core (50*128 = 6400)
NP = NB // 2                 # DoubleRow bucket pairs = 25
VSH = NB * 128               # vocab shard size = 6400
P_B = 256                    # token slots per (bucket, row-half) cell
TPB = 4                      # tiles per bucket = 2 halves x 2 tiles
NT = NB * TPB                # token tiles per core = 200
SPILL = 128                  # overflow token slots per core
WSCALE = 1024.0              # fp8 weight scale S (descaled at h1p copy)

_CACHE = {}


def _build(reps=1, sim=False, nocc=False, ablate=None):
    # sim: single-core build with the collective stubbed (for TimelineSim).
    # nocc: 8-core build with the collective stubbed (HW contention probe).
    # ablate: None | 'stream' | 'stage1' | 'nos2tail' (HW bottleneck probes).
    nc = bacc.Bacc("TRN2", target_bir_lowering=False, debug=False,
                   num_devices=1 if sim else N_CORES)
    sim = sim or nocc
    f32 = mybir.dt.float32
    bf16 = mybir.dt.bfloat16
    f8 = mybir.dt.float8e4

    w1a = nc.dram_tensor("w1a", [128, NB, M1], f8, kind="ExternalInput")
    w1b = nc.dram_tensor("w1b", [128, NB, M1], f8, kind="ExternalInput")
    w1s = nc.dram_tensor("w1s", [VSH, M1], bf16, kind="ExternalInput")
    # packed f32 consts: rv | rw  -> [128, 2*NT]
    NCF = NT + NT
    cfp = nc.dram_tensor("cfp", [128, NCF], f32, kind="ExternalInput")
    msph = nc.dram_tensor("msph", [128, 256], f8, kind="ExternalInput")
    w2t = nc.dram_tensor("w2t", [128, M1 // 128, M2], bf16, kind="ExternalInput")
    w3t = nc.dram_tensor("w3t", [128, M2 // 128, EMB], bf16, kind="ExternalInput")
    # packed bf16 row consts: b1 | b2 | b3  -> [1, M1+M2+EMB]
    brow = nc.dram_tensor("brow", [1, M1 + M2 + EMB], bf16, kind="ExternalInput")
    s4id = nc.dram_tensor("s4id", [128, RPC], bf16, kind="ExternalInput")
    spidx = nc.dram_tensor("spidx", [128, 1], mybir.dt.int32, kind="ExternalInput")
    emb = nc.dram_tensor("emb", [reps * RPC, EMB], f32, kind="ExternalOutput")

    with tile.TileContext(nc) as tc:
        with (
            tc.tile_pool(name="const", bufs=1) as cpool,
            tc.tile_pool(name="etab", bufs=1) as epool,
            tc.tile_pool(name="bowsb", bufs=2) as bpool,
            tc.tile_pool(name="act", bufs=2) as apool,
            tc.tile_pool(name="oh", bufs=16) as ohpool,
            tc.tile_pool(name="pbow", bufs=2, space="PSUM") as pbow_pool,
            tc.tile_pool(name="ph1", bufs=1, space="PSUM") as ph1_pool,
            tc.tile_pool(name="ptail", bufs=1, space="PSUM") as ptail_pool,
            tc.tile_pool(name="dram", bufs=2, space="DRAM") as dpool,
        ):
            # ---- constants: rv/rw first (DVE needs them immediately);
            # everything else is emitted inside _body interleaved with the
            # weight stream so HWDGE/queue order favors the pipeline ----
            cf_t = cpool.tile([128, NCF], f32)
            nc.sync.dma_start(cf_t[:], cfp[:])
            rv_t = cf_t[:, 0:NT]
            rw_t = cf_t[:, NT:2 * NT]
            iota_i = cpool.tile([128, 128], mybir.dt.int32)
            nc.gpsimd.iota(iota_i[:], pattern=[[1, 128]], base=0, channel_multiplier=0)
            iotaR = cpool.tile([128, 128], bf16)
            nc.vector.tensor_copy(iotaR[:], iota_i[:])
            ones1 = cpool.tile([1, RPC], bf16)
            nc.gpsimd.memset(ones1[:], 1.0)
            spidx_t = cpool.tile([128, 1], mybir.dt.int32)
            nc.sync.dma_start(spidx_t[:], spidx[:])
            gsp = cpool.tile([128, M1], bf16)
            nc.gpsimd.indirect_dma_start(
                out=gsp[:], out_offset=None, in_=w1s[:],
                in_offset=bass.IndirectOffsetOnAxis(ap=spidx_t[:, 0:1], axis=0),
            )
            msp = cpool.tile([128, 256], f8)
            s4_t = cpool.tile([128, RPC], bf16)
            br_t = cpool.tile([1, M1 + M2 + EMB], bf16)
            b1_t = br_t[:, 0:M1]
            b2_t = br_t[:, M1:M1 + M2]
            b3_t = br_t[:, M1 + M2:M1 + M2 + EMB]
            w2_t = cpool.tile([128, M1 // 128, M2], bf16)
            w3_t = cpool.tile([128, M2 // 128, EMB], bf16)

            def load_consts_mid():
                nc.sync.dma_start(msp[:], msph[:])
                nc.sync.dma_start(s4_t[:], s4id[:])
                nc.sync.dma_start(br_t[:], brow[:])

            def load_consts_late():
                nc.sync.dma_start(w2_t[:], w2t[:])
                nc.sync.dma_start(w3_t[:], w3t[:])

            # 1-rep software pipeline: rep N's tail (collective + fc2/fc3)
            # is EMITTED after rep N+1's main, so no engine queue has a
            # tail instruction blocking the next rep's main work
            # (head-of-line ordering is what serialized reps on HW).
            pend = None
            for _rep in range(reps):
                h1p = _main(nc, tc, epool, bpool, apool, ohpool,
                            pbow_pool, ph1_pool,
                            gsp, msp, rv_t, rw_t, iotaR,
                            w1a, w1b,
                            load_consts_mid if _rep == 0 else None,
                            load_consts_late if _rep == 0 else None,
                            ablate)
                if pend is not None:
                    _tail(nc, apool, ptail_pool, dpool, pend,
                          w2_t, w3_t, b1_t, b2_t, b3_t, s4_t, ones1,
                          emb[(_rep - 1) * RPC:_rep * RPC, :], sim)
                if h1p is None and ablate is not None:
                    # ablation mode: dummy output, no tail
                    dummy = apool.tile([RPC, EMB], f32, tag="out")
                    nc.gpsimd.memset(dummy[:], 0.0)
                    nc.sync.dma_start(emb[_rep * RPC:(_rep + 1) * RPC, :],
                                      dummy[:])
                pend = h1p
            if pend is not None:
                _tail(nc, apool, ptail_pool, dpool, pend,
                      w2_t, w3_t, b1_t, b2_t, b3_t, s4_t, ones1,
                      emb[(reps - 1) * RPC:reps * RPC, :], sim)

    nc.compile()
    return nc


def _main(nc, tc, epool, bpool, apool, ohpool, pbow_pool, ph1_pool,
          gsp, msp, rv_t, rw_t, iotaR, w1a, w1b,
          load_consts_mid=None, load_consts_late=None, ablate=None):
    """Stream + histogram + DoubleRow fc1 + PSUM->bf16 casts.

    Returns the h1p tile (S-scaled bf16 partial h1), or None in ablation
    modes that stop early.
    """
    f32 = mybir.dt.float32
    bf16 = mybir.dt.bfloat16
    f8 = mybir.dt.float8e4
    Copy = mybir.ActivationFunctionType.Copy
    DR = mybir.MatmulPerfMode.DoubleRow
    eq = mybir.AluOpType.is_equal

    # ---- stream both fp8 weight tables in big chunks (HWDGE holds each
    # DMACopy ~600ns regardless of size, so few big transfers win);
    # small consts slot in after the first chunk, w2/w3 after the stream ----
    etA = epool.tile([128, NB, M1], f8, tag="etA")
    etB = epool.tile([128, NB, M1], f8, tag="etB")
    CHK = 5                                    # buckets per stream DMA
    for g in range(NB // CHK):
        sl = slice(g * CHK, (g + 1) * CHK)
        nc.sync.dma_start(etA[:, sl, :], w1a[:, sl, :])
        nc.sync.dma_start(etB[:, sl, :], w1b[:, sl, :])
        if g == 0 and load_consts_mid is not None:
            load_consts_mid()
    if load_consts_late is not None:
        load_consts_late()

    # ---- stage 1 (histogram) + stage 2 (DoubleRow fc1), single pass ----
    bowT = bpool.tile([128, NB, 256], f8, tag="bowT")
    ph1 = ph1_pool.tile([128, 2, 2, 512], f32, tag="h1")

    def stage1(q):
        pb = pbow_pool.tile([128, 256], f32, tag="bow")
        for j in range(TPB):
            t = q * TPB + j
            h = j // 2
            rt = ohpool.tile([128, 128], bf16, tag="oh")
            nc.vector.tensor_scalar(rt[:], iotaR[:], rv_t[:, t:t + 1], None, op0=eq)
            rm = ohpool.tile([128, 128], bf16, tag="oh")
            nc.vector.tensor_scalar(rm[:], iotaR[:], rw_t[:, t:t + 1], None, op0=eq)
            nc.tensor.matmul(pb[:, h * 128:(h + 1) * 128], lhsT=rt[:], rhs=rm[:],
                             start=(j % 2 == 0), stop=(j % 2 == 1))
        nc.scalar.activation(bowT[:, q, :], pb[:], Copy)

    def stage2(p):
        for h in range(2):
            for m in range(2):
                nc.tensor.matmul(
                    ph1[:, h, m, :],
                    lhsT=bowT[:, 2 * p:2 * p + 2, h * 128:(h + 1) * 128],
                    rhs=etA[:, 2 * p:2 * p + 2, m * 512:(m + 1) * 512],
                    start=(p == 0), stop=False, perf_mode=DR)
                nc.tensor.matmul(
                    ph1[:, h, m, :],
                    lhsT=bowT[:, 2 * p:2 * p + 2, h * 128:(h + 1) * 128],
                    rhs=etB[:, 2 * p:2 * p + 2, m * 512:(m + 1) * 512],
                    start=False, stop=False, perf_mode=DR)

    if ablate == "stream":
        return None
    for p in range(NP):
        stage1(2 * p)
        stage1(2 * p + 1)
        if p >= 1 and ablate != "stage1":
            stage2(p - 1)
    if ablate == "stage1":
        return None
    # final pair + spill, finished REGION BY REGION so the PSUM -> bf16 cast
    # copies (split across Scalar + DVE; 1/S folds into the s4 selector)
    # overlap the remaining matmuls
    h1p = apool.tile([128, 2, 2, 512], bf16, tag="h1p")
    pl = NP - 1
    for i, (h, m) in enumerate(((0, 0), (0, 1), (1, 0), (1, 1))):
        nc.tensor.matmul(
            ph1[:, h, m, :],
            lhsT=bowT[:, 2 * pl:2 * pl + 2, h * 128:(h + 1) * 128],
            rhs=etA[:, 2 * pl:2 * pl + 2, m * 512:(m + 1) * 512],
            start=False, stop=False, perf_mode=DR)
        nc.tensor.matmul(
            ph1[:, h, m, :],
            lhsT=bowT[:, 2 * pl:2 * pl + 2, h * 128:(h + 1) * 128],
            rhs=etB[:, 2 * pl:2 * pl + 2, m * 512:(m + 1) * 512],
            start=False, stop=False, perf_mode=DR)
        # spill contribution (S-scaled bf16 rows, exact) ends this region
        nc.tensor.matmul(ph1[:, h, m, :], lhsT=msp[:, h * 128:(h + 1) * 128],
                         rhs=gsp[:, m * 512:(m + 1) * 512],
                         start=False, stop=True)
        if i % 2 == 0:
            nc.scalar.activation(h1p[:, h, m, :], ph1[:, h, m, :], Copy)
        else:
            nc.vector.tensor_copy(h1p[:, h, m, :], ph1[:, h, m, :])
    if ablate == "nos2tail":
        return None
    return h1p


def _tail(nc, apool, ptail_pool, dpool, h1p,
          w2_t, w3_t, b1_t, b2_t, b3_t, s4_t, ones1, emb, sim=False):
    """Exchange + partial-sum + fc2/fc3.  All DMA legs ride the Pool queue
    (alongside the collective) so the SP stream queue never blocks on a
    collective, and tail matmuls are emitted one rep late (see _build)."""
    f32 = mybir.dt.float32
    bf16 = mybir.dt.bfloat16
    Relu = mybir.ActivationFunctionType.Relu

    # ---- exchange partial h1: AllToAll (8 chunks of 32 rows) ----
    cc_in = dpool.tile([B, M1], bf16, tag="cc_in")
    cc_out = dpool.tile([B, M1], bf16, tag="cc_out")
    ccv = cc_in[:].rearrange("(h p) (b m) -> h p b m", p=128, b=2)
    nc.gpsimd.dma_start(ccv[0], h1p[:, 0])
    nc.gpsimd.dma_start(ccv[1], h1p[:, 1])
    if sim:
        nc.gpsimd.dma_start(cc_out[:], cc_in[:])
    else:
        nc.gpsimd.collective_compute(
            "AllToAll", mybir.AluOpType.bypass,
            replica_groups=[list(range(N_CORES))],
            ins=[cc_in[:]], outs=[cc_out[:]],
        )
    # read back in two M1-halves so the partial-sum matmuls start while the
    # second half is still in flight
    cc_sb = apool.tile([128, 2, M1], bf16, tag="ccsb")
    ccov = cc_out[:].rearrange("(d q r) (x m) -> x (q r) d m", d=2, q=4, x=2)
    nc.gpsimd.dma_start(cc_sb[:, :, 0:M1 // 2], ccov[0])
    nc.gpsimd.dma_start(cc_sb[:, :, M1 // 2:M1], ccov[1])

    # ---- sum the 8 partials on TensorE, TRANSPOSED: h1T [feat128, a, rows];
    # s4 selector carries 1/S; b1 folds in via a K=1 ones-matmul so the
    # relu is ONE wide activation, not 8 per-bias ops ----
    pt1 = ptail_pool.tile([128, M1 // 128, RPC], f32, tag="tail")
    for a in range(M1 // 128):
        for d in range(2):
            nc.tensor.matmul(pt1[:, a, :], lhsT=cc_sb[:, d, a * 128:(a + 1) * 128],
                             rhs=s4_t[:], start=(d == 0), stop=False)
        nc.tensor.matmul(pt1[:, a, :], lhsT=b1_t[:, a * 128:(a + 1) * 128],
                         rhs=ones1[:], start=False, stop=True)
    h1T = apool.tile([128, M1 // 128, RPC], bf16, tag="h1T")
    nc.scalar.activation(h1T[:], pt1[:], Relu)

    # ---- fc2, output transposed: h2T [feat128, m4, rows] ----
    pt2 = ptail_pool.tile([128, M1 // 128, RPC], f32, tag="tail")
    for m4 in range(M2 // 128):
        for a in range(M1 // 128):
            nc.tensor.matmul(pt2[:, m4, :], lhsT=w2_t[:, a, m4 * 128:(m4 + 1) * 128],
                             rhs=h1T[:, a, :],
                             start=(a == 0), stop=False)
        nc.tensor.matmul(pt2[:, m4, :], lhsT=b2_t[:, m4 * 128:(m4 + 1) * 128],
                         rhs=ones1[:], start=False, stop=True)
    h2T = apool.tile([128, M2 // 128, RPC], bf16, tag="h2T")
    nc.scalar.activation(h2T[:], pt2[:, 0:M2 // 128, :], Relu)

    # ---- fc3, row-major output [32, 256] ----
    pt3f = ptail_pool.tile([128, M1 // 128, RPC], f32, tag="tail")
    pt3 = pt3f[0:RPC, 0:EMB // RPC, :]
    for m4 in range(M2 // 128):
        nc.tensor.matmul(pt3, lhsT=h2T[:, m4, :], rhs=w3_t[:, m4, :],
                         start=(m4 == 0), stop=False)
    nc.tensor.matmul(pt3, lhsT=ones1[:], rhs=b3_t[:], start=False, stop=True)
    out_t = apool.tile([RPC, EMB], f32, tag="out")
    nc.scalar.activation(out_t[:], pt3, Relu)
    nc.gpsimd.dma_start(emb[:], out_t[:])


def _prep_inputs(idx, W1, b1, W2, b2, W3, b3):
    """Host-side sharding/layout prep (index routing + dtype/layout only)."""
    import ml_dtypes

    bf16 = ml_dtypes.bfloat16
    f8np = mybir.dt.np(mybir.dt.float8e4)
    idx = np.asarray(idx).astype(np.int64)
    VPAD = N_CORES * VSH
    w1f = np.zeros((VPAD, M1), dtype=np.float32)
    w1f[:V] = np.asarray(W1, dtype=np.float32).T
    w1f *= WSCALE
    w1A = w1f.astype(f8np)
    w1B = (w1f - w1A.astype(np.float32)).astype(f8np)
    w1sc = w1f.astype(bf16)          # S-scaled bf16 rows for the spill gather

    w2t = np.ascontiguousarray(
        np.asarray(W2, dtype=np.float32).T.reshape(M1 // 128, 128, M2)
        .transpose(1, 0, 2)).astype(bf16)
    w3t = np.ascontiguousarray(
        np.asarray(W3, dtype=np.float32).T.reshape(M2 // 128, 128, EMB)
        .transpose(1, 0, 2)).astype(bf16)
    browp = np.concatenate([
        np.asarray(b1, dtype=np.float32),
        np.asarray(b2, dtype=np.float32),
        np.asarray(b3, dtype=np.float32)]).reshape(1, -1).astype(bf16)
    s4id = ((np.arange(128)[:, None] % RPC == np.arange(RPC)[None, :])
            .astype(np.float32) / WSCALE).astype(bf16)

    rows = np.repeat(np.arange(B, dtype=np.int64), S)
    vals = idx.reshape(-1)
    core = vals // VSH
    in_maps = []
    for c in range(N_CORES):
        sel = core == c
        v = vals[sel] - c * VSH
        r = rows[sel]
        q = v // 128
        rl = v % 128
        order = np.argsort(q, kind="stable")
        q, rl, r, v = q[order], rl[order], r[order], v[order]

        rv_arr = np.full((NT * 128,), 200, dtype=np.int64)
        rw_arr = np.full((NT * 128,), 300, dtype=np.int64)
        sp_idx = np.zeros((SPILL,), dtype=np.int32)
        sp_row = np.full((SPILL,), 300, dtype=np.int64)
        n_spill = 0
        for qq in range(NB):
            for hh in range(2):
                m = (q == qq) & ((r // 128) == hh)
                nq = int(m.sum())
                take = min(nq, P_B)
                base = (qq * 4 + hh * 2) * 128
                rv_arr[base:base + take] = rl[m][:take]
                rw_arr[base:base + take] = r[m][:take] % 128
                if nq > take:
                    ov = nq - take
                    assert n_spill + ov <= SPILL, "spill capacity exceeded"
                    sp_idx[n_spill:n_spill + ov] = v[m][take:]
                    sp_row[n_spill:n_spill + ov] = r[m][take:]
                    n_spill += ov
        rv_til = rv_arr.reshape(NT, 128).T        # [128, NT]
        rw_til = rw_arr.reshape(NT, 128).T
        cfpk = np.ascontiguousarray(np.concatenate(
            [rv_til, rw_til], axis=1).astype(np.float32))
        mspa = (sp_row[:, None] == np.arange(256)[None, :]).astype(f8np)

        w1Ac = w1A[c * VSH:(c + 1) * VSH]                     # [6400, 1024]
        w1Bc = w1B[c * VSH:(c + 1) * VSH]
        w1atl = np.ascontiguousarray(
            w1Ac.reshape(NB, 128, M1).transpose(1, 0, 2))     # [128, 50, 1024]
        w1btl = np.ascontiguousarray(
            w1Bc.reshape(NB, 128, M1).transpose(1, 0, 2))

        in_maps.append({
            "w1a": w1atl,
            "w1b": w1btl,
            "w1s": np.ascontiguousarray(w1sc[c * VSH:(c + 1) * VSH]),
            "cfp": cfpk,
            "msph": mspa,
            "w2t": w2t, "w3t": w3t,
            "brow": browp, "s4id": s4id,
            "spidx": sp_idx.reshape(128, 1),
        })
    return in_maps


def kernel(idx, W1, b1, W2, b2, W3, b3):
    if "nc" not in _CACHE:
        _CACHE["nc"] = _build()
    nc = _CACHE["nc"]
    in_maps = _prep_inputs(idx, W1, b1, W2, b2, W3, b3)
    try:
        res = run_bass_kernel_spmd(nc, in_maps, list(range(N_CORES)))
    except Exception:
        res = run_bass_kernel_spmd(nc, in_maps, list(range(N_CORES)))
    return np.concatenate([res.results[c]["emb"] for c in range(N_CORES)], axis=0)


# revision 39
# speedup vs baseline: 1.3195x; 1.2557x over previous
"""Trainium2 Bass kernel for nn_BOW (bag-of-words MLP).

emb = relu(relu(relu(bow(idx) @ W1.T + b1) @ W2.T + b2) @ W3.T + b3)

Design (8 NeuronCores):

fc1 is sharded over the vocab axis: core c owns rows [c*6400, (c+1)*6400) of
W1T (50000 padded to 51200).  W1 is stored as an fp8e4m3 two-term residual
split A + B (A = e4m3(S*W1), B = e4m3(S*W1 - A), shared scale S = 2^10),
which matches bf16 end-to-end accuracy while letting fc1 run in DoubleRow
fp8 perf mode: each matmul contracts TWO 128-lane vocab buckets per pass
(adjacent bucket pairs of the same table), so the dense fc1 runs at 2x bf16
MAC throughput while streaming the same 12.8 MB/core of weights.

Histogram without scatter: tokens are host-routed to (core, vocab-bucket of
128, batch-row-half) slots.  For each 128-token tile, one TensorE matmul
R^T @ M accumulates exact counts into PSUM, where R (token -> vocab lane)
and M (token -> row) are one-hot matrices built on DVE by iota-compare from
tiny per-slot id vectors (rv, rw) -- no one-hot streaming from HBM.  The
PSUM counts are copied to fp8 bowT on the Scalar engine (counts <= 15 are
exact in e4m3), feeding stage-2 DoubleRow matmuls as lhsT bucket pairs.

Partial h1 [256, 1024] (S-scaled bf16; 1/S folds into the stacked-identity
selector) is exchanged with a single AllToAll and summed on TensorE with a
matmul whose output is ALREADY transposed (features on partitions), so the
bf16 fc2/fc3 tail needs no PE transposes; b1/b2/b3 fold in via K=1
ones-vector matmuls so each relu is one wide activation.  A tiny
indirect-DMA gather path (128 slots, S-scaled bf16 rows) handles
bucket-capacity overflow exactly.  Host-side prep is index routing, dtype
casts and relayout only; all model arithmetic runs on device.

Scheduling: engines execute their queues IN ORDER, so rep N's tail
(exchange + fc2/fc3) is emitted one rep late and its DMA legs ride the
Pool/SWDGE queue -- no tail instruction ever sits between two reps' main
work on the SP/DVE/Act/PE queues (head-of-line blocking there serialized
whole reps: measured +65 us/rep).  Stream DMAs are issued in 640 KB chunks
because the shared HWDGE descriptor unit costs ~625 ns per DMACopy
regardless of size.
"""
import os, sys

os.environ.setdefault("JAX_PLATFORMS", "cpu,axon")
try:
    import concourse.bass  # noqa: F401
except ImportError:
    sys.path.insert(0, "/opt/trn_rl_repo")

import numpy as np
import concourse.bass as bass
import concourse.tile as tile
import concourse.mybir as mybir
from concourse import bacc
from concourse.bass_utils import run_bass_kernel_spmd

N_CORES = 8
B, S = 256, 512
V = 50000
M1, M2, EMB = 1024, 512, 256
RPC = B // N_CORES           # rows per core after the exchange = 32
NB = 50                      # vocab buckets per core (50*128 = 6400)
NP = NB // 2                 # DoubleRow bucket pairs = 25
VSH = NB * 128               # vocab shard size = 6400
P_B = 256                    # token slots per (bucket, row-half) cell
TPB = 4                      # tiles per bucket = 2 halves x 2 tiles
NT = NB * TPB                # token tiles per core = 200
SPILL = 128                  # overflow token slots per core
WSCALE = 1024.0              # fp8 weight scale S (descaled at h1p copy)

_CACHE = {}


def _build(reps=1, sim=False, nocc=False, ablate=None):
    # sim: single-core build with the collective stubbed (for TimelineSim).
    # nocc: 8-core build with the collective stubbed (HW contention probe).
    # ablate: None | 'stream' | 'stage1' | 'nos2tail' (HW bottleneck probes).
    nc = bacc.Bacc("TRN2", target_bir_lowering=False, debug=False,
                   num_devices=1 if sim else N_CORES)
    sim = sim or nocc
    f32 = mybir.dt.float32
    bf16 = mybir.dt.bfloat16
    f8 = mybir.dt.float8e4

    w1a = nc.dram_tensor("w1a", [128, NB, M1], f8, kind="ExternalInput")
    w1b = nc.dram_tensor("w1b", [128, NB, M1], f8, kind="ExternalInput")
    w1s = nc.dram_tensor("w1s", [VSH, M1], bf16, kind="ExternalInput")
    # packed f32 consts: rv | rw  -> [128, 2*NT]
    NCF = NT + NT
    cfp = nc.dram_tensor("cfp", [128, NCF], f32, kind="ExternalInput")
    msph = nc.dram_tensor("msph", [128, 256], f8, kind="ExternalInput")
    w2t = nc.dram_tensor("w2t", [128, M1 // 128, M2], bf16, kind="ExternalInput")
    w3t = nc.dram_tensor("w3t", [128, M2 // 128, EMB], bf16, kind="ExternalInput")
    # packed bf16 row consts: b1 | b2 | b3  -> [1, M1+M2+EMB]
    brow = nc.dram_tensor("brow", [1, M1 + M2 + EMB], bf16, kind="ExternalInput")
    s4id = nc.dram_tensor("s4id", [128, RPC], bf16, kind="ExternalInput")
    spidx = nc.dram_tensor("spidx", [128, 1], mybir.dt.int32, kind="ExternalInput")
    emb = nc.dram_tensor("emb", [reps * RPC, EMB], f32, kind="ExternalOutput")

    with tile.TileContext(nc) as tc:
        with (
            tc.tile_pool(name="const", bufs=1) as cpool,
            tc.tile_pool(name="etab", bufs=1) as epool,
            tc.tile_pool(name="bowsb", bufs=2) as bpool,
            tc.tile_pool(name="act", bufs=2) as apool,
            tc.tile_pool(name="oh", bufs=24) as ohpool,
            tc.tile_pool(name="pbow", bufs=2, space="PSUM") as pbow_pool,
            tc.tile_pool(name="ph1", bufs=1, space="PSUM") as ph1_pool,
            tc.tile_pool(name="ptail", bufs=1, space="PSUM") as ptail_pool,
            tc.tile_pool(name="dram", bufs=2, space="DRAM") as dpool,
        ):
            # ---- constants: rv/rw first (DVE needs them immediately);
            # everything else is emitted inside _body interleaved with the
            # weight stream so HWDGE/queue order favors the pipeline ----
            cf_t = cpool.tile([128, NCF], f32)
            nc.sync.dma_start(cf_t[:], cfp[:])
            rv_t = cf_t[:, 0:NT]
            rw_t = cf_t[:, NT:2 * NT]
            iota_i = cpool.tile([128, 128], mybir.dt.int32)
            nc.gpsimd.iota(iota_i[:], pattern=[[1, 128]], base=0, channel_multiplier=0)
            iotaR = cpool.tile([128, 128], bf16)
            nc.vector.tensor_copy(iotaR[:], iota_i[:])
            ones1 = cpool.tile([1, RPC], bf16)
            nc.gpsimd.memset(ones1[:], 1.0)
            spidx_t = cpool.tile([128, 1], mybir.dt.int32)
            nc.sync.dma_start(spidx_t[:], spidx[:])
            gsp = cpool.tile([128, M1], bf16)
            nc.gpsimd.indirect_dma_start(
                out=gsp[:], out_offset=None, in_=w1s[:],
                in_offset=bass.IndirectOffsetOnAxis(ap=spidx_t[:, 0:1], axis=0),
            )
            msp = cpool.tile([128, 256], f8)
            s4_t = cpool.tile([128, RPC], bf16)
            br_t = cpool.tile([1, M1 + M2 + EMB], bf16)
            b1_t = br_t[:, 0:M1]
            b2_t = br_t[:, M1:M1 + M2]
            b3_t = br_t[:, M1 + M2:M1 + M2 + EMB]
            w2_t = cpool.tile([128, M1 // 128, M2], bf16)
            w3_t = cpool.tile([128, M2 // 128, EMB], bf16)

            def load_consts_mid():
                nc.sync.dma_start(msp[:], msph[:])
                nc.sync.dma_start(s4_t[:], s4id[:])
                nc.sync.dma_start(br_t[:], brow[:])

            def load_consts_late():
                nc.sync.dma_start(w2_t[:], w2t[:])
                nc.sync.dma_start(w3_t[:], w3t[:])

            # 1-rep software pipeline: rep N's tail (collective + fc2/fc3)
            # is EMITTED after rep N+1's main, so no engine queue has a
            # tail instruction blocking the next rep's main work
            # (head-of-line ordering is what serialized reps on HW).
            pend = None
            for _rep in range(reps):
                h1p = _main(nc, tc, epool, bpool, apool, ohpool,
                            pbow_pool, ph1_pool,
                            gsp, msp, rv_t, rw_t, iotaR,
                            w1a, w1b,
                            load_consts_mid if _rep == 0 else None,
                            load_consts_late if _rep == 0 else None,
                            ablate)
                if pend is not None:
                    _tail(nc, apool, ptail_pool, dpool, pend,
                          w2_t, w3_t, b1_t, b2_t, b3_t, s4_t, ones1,
                          emb[(_rep - 1) * RPC:_rep * RPC, :], sim)
                if h1p is None and ablate is not None:
                    # ablation mode: dummy output, no tail
                    dummy = apool.tile([RPC, EMB], f32, tag="out")
                    nc.gpsimd.memset(dummy[:], 0.0)
                    nc.sync.dma_start(emb[_rep * RPC:(_rep + 1) * RPC, :],
                                      dummy[:])
                pend = h1p
            if pend is not None:
                _tail(nc, apool, ptail_pool, dpool, pend,
                      w2_t, w3_t, b1_t, b2_t, b3_t, s4_t, ones1,
                      emb[(reps - 1) * RPC:reps * RPC, :], sim)

    nc.compile()
    return nc


def _main(nc, tc, epool, bpool, apool, ohpool, pbow_pool, ph1_pool,
          gsp, msp, rv_t, rw_t, iotaR, w1a, w1b,
          load_consts_mid=None, load_consts_late=None, ablate=None):
    """Stream + histogram + DoubleRow fc1 + PSUM->bf16 casts.

    Returns the h1p tile (S-scaled bf16 partial h1), or None in ablation
    modes that stop early.
    """
    f32 = mybir.dt.float32
    bf16 = mybir.dt.bfloat16
    f8 = mybir.dt.float8e4
    Copy = mybir.ActivationFunctionType.Copy
    DR = mybir.MatmulPerfMode.DoubleRow
    eq = mybir.AluOpType.is_equal

    # ---- stream both fp8 weight tables in big chunks (HWDGE holds each
    # DMACopy ~600ns regardless of size, so few big transfers win);
    # small consts slot in after the first chunk, w2/w3 after the stream ----
    etA = epool.tile([128, NB, M1], f8, tag="etA")
    etB = epool.tile([128, NB, M1], f8, tag="etB")
    CHK = 4                                    # buckets per stream DMA;
    # multiple of 2 so DoubleRow bucket pairs never span a chunk boundary
    bounds = list(range(0, NB, CHK)) + [NB]
    for g in range(len(bounds) - 1):
        sl = slice(bounds[g], bounds[g + 1])
        nc.sync.dma_start(etA[:, sl, :], w1a[:, sl, :])
        nc.sync.dma_start(etB[:, sl, :], w1b[:, sl, :])
        if g == 0 and load_consts_mid is not None:
            load_consts_mid()
    if load_consts_late is not None:
        load_consts_late()

    # ---- stage 1 (histogram) + stage 2 (DoubleRow fc1), single pass ----
    bowT = bpool.tile([128, NB, 256], f8, tag="bowT")
    ph1 = ph1_pool.tile([128, 2, 2, 512], f32, tag="h1")

    def stage1(q):
        pb = pbow_pool.tile([128, 256], f32, tag="bow")
        for j in range(TPB):
            t = q * TPB + j
            h = j // 2
            rt = ohpool.tile([128, 128], bf16, tag="oh")
            nc.vector.tensor_scalar(rt[:], iotaR[:], rv_t[:, t:t + 1], None, op0=eq)
            rm = ohpool.tile([128, 128], bf16, tag="oh")
            nc.vector.tensor_scalar(rm[:], iotaR[:], rw_t[:, t:t + 1], None, op0=eq)
            nc.tensor.matmul(pb[:, h * 128:(h + 1) * 128], lhsT=rt[:], rhs=rm[:],
                             start=(j % 2 == 0), stop=(j % 2 == 1))
        nc.scalar.activation(bowT[:, q, :], pb[:], Copy)

    def stage2(p):
        for h in range(2):
            for m in range(2):
                nc.tensor.matmul(
                    ph1[:, h, m, :],
                    lhsT=bowT[:, 2 * p:2 * p + 2, h * 128:(h + 1) * 128],
                    rhs=etA[:, 2 * p:2 * p + 2, m * 512:(m + 1) * 512],
                    start=(p == 0), stop=False, perf_mode=DR)
                nc.tensor.matmul(
                    ph1[:, h, m, :],
                    lhsT=bowT[:, 2 * p:2 * p + 2, h * 128:(h + 1) * 128],
                    rhs=etB[:, 2 * p:2 * p + 2, m * 512:(m + 1) * 512],
                    start=False, stop=False, perf_mode=DR)

    if ablate == "stream":
        return None
    for p in range(NP):
        stage1(2 * p)
        stage1(2 * p + 1)
        if p >= 1 and ablate != "stage1":
            stage2(p - 1)
    if ablate == "stage1":
        return None
    # final pair + spill, finished REGION BY REGION so the PSUM -> bf16 cast
    # copies (split across Scalar + DVE; 1/S folds into the s4 selector)
    # overlap the remaining matmuls
    h1p = apool.tile([128, 2, 2, 512], bf16, tag="h1p")
    pl = NP - 1
    for i, (h, m) in enumerate(((0, 0), (0, 1), (1, 0), (1, 1))):
        nc.tensor.matmul(
            ph1[:, h, m, :],
            lhsT=bowT[:, 2 * pl:2 * pl + 2, h * 128:(h + 1) * 128],
            rhs=etA[:, 2 * pl:2 * pl + 2, m * 512:(m + 1) * 512],
            start=False, stop=False, perf_mode=DR)
        nc.tensor.matmul(
            ph1[:, h, m, :],
            lhsT=bowT[:, 2 * pl:2 * pl + 2, h * 128:(h + 1) * 128],
            rhs=etB[:, 2 * pl:2 * pl + 2, m * 512:(m + 1) * 512],
            start=False, stop=False, perf_mode=DR)
        # spill contribution (S-scaled bf16 rows, exact) ends this region
        nc.tensor.matmul(ph1[:, h, m, :], lhsT=msp[:, h * 128:(h + 1) * 128],
                         rhs=gsp[:, m * 512:(m + 1) * 512],
                         start=False, stop=True)
        if i % 2 == 0:
            nc.scalar.activation(h1p[:, h, m, :], ph1[:, h, m, :], Copy)
        else:
            nc.vector.tensor_copy(h1p[:, h, m, :], ph1[:, h, m, :])
    if ablate == "nos2tail":
        return None
    return h1p


def _tail(nc, apool, ptail_pool, dpool, h1p,
          w2_t, w3_t, b1_t, b2_t, b3_t, s4_t, ones1, emb, sim=False):
    """Exchange + partial-sum + fc2/fc3.  All DMA legs ride the Pool queue
    (alongside the collective) so the SP stream queue never blocks on a
    collective, and tail matmuls are emitted one rep late (see _build)."""
    f32 = mybir.dt.float32
    bf16 = mybir.dt.bfloat16
    Relu = mybir.ActivationFunctionType.Relu

    # ---- exchange partial h1: AllToAll (8 chunks of 32 rows) ----
    cc_in = dpool.tile([B, M1], bf16, tag="cc_in")
    cc_out = dpool.tile([B, M1], bf16, tag="cc_out")
    ccv = cc_in[:].rearrange("(h p) (b m) -> h p b m", p=128, b=2)
    nc.gpsimd.dma_start(ccv[0], h1p[:, 0])
    nc.gpsimd.dma_start(ccv[1], h1p[:, 1])
    if sim:
        nc.gpsimd.dma_start(cc_out[:], cc_in[:])
    else:
        nc.gpsimd.collective_compute(
            "AllToAll", mybir.AluOpType.bypass,
            replica_groups=[list(range(N_CORES))],
            ins=[cc_in[:]], outs=[cc_out[:]],
        )
    # read back in two M1-halves so the partial-sum matmuls start while the
    # second half is still in flight
    cc_sb = apool.tile([128, 2, M1], bf16, tag="ccsb")
    ccov = cc_out[:].rearrange("(d q r) (x m) -> x (q r) d m", d=2, q=4, x=2)
    nc.gpsimd.dma_start(cc_sb[:, :, 0:M1 // 2], ccov[0])
    nc.gpsimd.dma_start(cc_sb[:, :, M1 // 2:M1], ccov[1])

    # ---- sum the 8 partials on TensorE, TRANSPOSED: h1T [feat128, a, rows];
    # s4 selector carries 1/S; b1 folds in via a K=1 ones-matmul so the
    # relu is ONE wide activation, not 8 per-bias ops ----
    pt1 = ptail_pool.tile([128, M1 // 128, RPC], f32, tag="tail")
    for a in range(M1 // 128):
        for d in range(2):
            nc.tensor.matmul(pt1[:, a, :], lhsT=cc_sb[:, d, a * 128:(a + 1) * 128],
                             rhs=s4_t[:], start=(d == 0), stop=False)
        nc.tensor.matmul(pt1[:, a, :], lhsT=b1_t[:, a * 128:(a + 1) * 128],
                         rhs=ones1[:], start=False, stop=True)
    h1T = apool.tile([128, M1 // 128, RPC], bf16, tag="h1T")
    nc.scalar.activation(h1T[:], pt1[:], Relu)

    # ---- fc2, output transposed: h2T [feat128, m4, rows] ----
    pt2 = ptail_pool.tile([128, M1 // 128, RPC], f32, tag="tail")
    for m4 in range(M2 // 128):
        for a in range(M1 // 128):
            nc.tensor.matmul(pt2[:, m4, :], lhsT=w2_t[:, a, m4 * 128:(m4 + 1) * 128],
                             rhs=h1T[:, a, :],
                             start=(a == 0), stop=False)
        nc.tensor.matmul(pt2[:, m4, :], lhsT=b2_t[:, m4 * 128:(m4 + 1) * 128],
                         rhs=ones1[:], start=False, stop=True)
    h2T = apool.tile([128, M2 // 128, RPC], bf16, tag="h2T")
    nc.scalar.activation(h2T[:], pt2[:, 0:M2 // 128, :], Relu)

    # ---- fc3, row-major output [32, 256] ----
    pt3f = ptail_pool.tile([128, M1 // 128, RPC], f32, tag="tail")
    pt3 = pt3f[0:RPC, 0:EMB // RPC, :]
    for m4 in range(M2 // 128):
        nc.tensor.matmul(pt3, lhsT=h2T[:, m4, :], rhs=w3_t[:, m4, :],
                         start=(m4 == 0), stop=False)
    nc.tensor.matmul(pt3, lhsT=ones1[:], rhs=b3_t[:], start=False, stop=True)
    out_t = apool.tile([RPC, EMB], f32, tag="out")
    nc.scalar.activation(out_t[:], pt3, Relu)
    nc.gpsimd.dma_start(emb[:], out_t[:])


def _prep_inputs(idx, W1, b1, W2, b2, W3, b3):
    """Host-side sharding/layout prep (index routing + dtype/layout only)."""
    import ml_dtypes

    bf16 = ml_dtypes.bfloat16
    f8np = mybir.dt.np(mybir.dt.float8e4)
    idx = np.asarray(idx).astype(np.int64)
    VPAD = N_CORES * VSH
    w1f = np.zeros((VPAD, M1), dtype=np.float32)
    w1f[:V] = np.asarray(W1, dtype=np.float32).T
    w1f *= WSCALE
    w1A = w1f.astype(f8np)
    w1B = (w1f - w1A.astype(np.float32)).astype(f8np)
    w1sc = w1f.astype(bf16)          # S-scaled bf16 rows for the spill gather

    w2t = np.ascontiguousarray(
        np.asarray(W2, dtype=np.float32).T.reshape(M1 // 128, 128, M2)
        .transpose(1, 0, 2)).astype(bf16)
    w3t = np.ascontiguousarray(
        np.asarray(W3, dtype=np.float32).T.reshape(M2 // 128, 128, EMB)
        .transpose(1, 0, 2)).astype(bf16)
    browp = np.concatenate([
        np.asarray(b1, dtype=np.float32),
        np.asarray(b2, dtype=np.float32),
        np.asarray(b3, dtype=np.float32)]).reshape(1, -1).astype(bf16)
    s4id = ((np.arange(128)[:, None] % RPC == np.arange(RPC)[None, :])
            .astype(np.float32) / WSCALE).astype(bf16)

    rows = np.repeat(np.arange(B, dtype=np.int64), S)
    vals = idx.reshape(-1)
    core = vals // VSH
    in_maps = []
    for c in range(N_CORES):
        sel = core == c
        v = vals[sel] - c * VSH
        r = rows[sel]
        q = v // 128
        rl = v % 128
        order = np.argsort(q, kind="stable")
        q, rl, r, v = q[order], rl[order], r[order], v[order]

        rv_arr = np.full((NT * 128,), 200, dtype=np.int64)
        rw_arr = np.full((NT * 128,), 300, dtype=np.int64)
        sp_idx = np.zeros((SPILL,), dtype=np.int32)
        sp_row = np.full((SPILL,), 300, dtype=np.int64)
        n_spill = 0
        for qq in range(NB):
            for hh in range(2):
                m = (q == qq) & ((r // 128) == hh)
                nq = int(m.sum())
                take = min(nq, P_B)
                base = (qq * 4 + hh * 2) * 128
                rv_arr[base:base + take] = rl[m][:take]
                rw_arr[base:base + take] = r[m][:take] % 128
                if nq > take:
                    ov = nq - take
                    assert n_spill + ov <= SPILL, "spill capacity exceeded"
                    sp_idx[n_spill:n_spill + ov] = v[m][take:]
                    sp_row[n_spill:n_spill + ov] = r[m][take:]
                    n_spill += ov
        rv_til = rv_arr.reshape(NT, 128).T        # [128, NT]
        rw_til = rw_arr.reshape(NT, 128).T
        cfpk = np.ascontiguousarray(np.concatenate(
            [rv_til, rw_til], axis=1).astype(np.float32))
        mspa = (sp_row[:, None] == np.arange(256)[None, :]).astype(f8np)

        w1Ac = w1A[c * VSH:(c + 1) * VSH]                     # [6400, 1024]
        w1Bc = w1B[c * VSH:(c + 1) * VSH]
        w1atl = np.ascontiguousarray(
            w1Ac.reshape(NB, 128, M1).transpose(1, 0, 2))     # [128, 50, 1024]
        w1btl = np.ascontiguousarray(
            w1Bc.reshape(NB, 128, M1).transpose(1, 0, 2))

        in_maps.append({
            "w1a": w1atl,
            "w1b": w1btl,
            "w1s": np.ascontiguousarray(w1sc[c * VSH:(c + 1) * VSH]),
            "cfp": cfpk,
            "msph": mspa,
            "w2t": w2t, "w3t": w3t,
            "brow": browp, "s4id": s4id,
            "spidx": sp_idx.reshape(128, 1),
        })
    return in_maps


def kernel(idx, W1, b1, W2, b2, W3, b3):
    if "nc" not in _CACHE:
        _CACHE["nc"] = _build()
    nc = _CACHE["nc"]
    in_maps = _prep_inputs(idx, W1, b1, W2, b2, W3, b3)
    try:
        res = run_bass_kernel_spmd(nc, in_maps, list(range(N_CORES)))
    except Exception:
        res = run_bass_kernel_spmd(nc, in_maps, list(range(N_CORES)))
    return np.concatenate([res.results[c]["emb"] for c in range(N_CORES)], axis=0)
